# revision 27
# baseline (speedup 1.0000x reference)
"""GAT (2-layer graph attention network) Bass kernel for 8 Trainium2 NeuronCores.

Strategy: edges partitioned by destination-node block (N/8 dst nodes per core,
blocks of 128). Layer-1 node features (h = x @ W1aug, alphas fused via an
augmented weight matrix) are computed replicated on every core into a DRAM
table; per-edge source rows arrive via SWDGE gather DMAs with runtime-exact
descriptor counts (slots are -1-padded; counts come from a per-core tensor via
reg_load). Self-loop edges never enter the gathers - they are applied from
SBUF-resident own-node tiles. Destination-side alphas are never gathered
either: they are broadcast across partitions with a PE transpose + rank-1
ones matmul and selected per edge with a masked DVE multiply+reduce against
the same one-hot mask used for aggregation. Segment softmax + weighted
aggregation run as PE matmuls with the one-hot {slot x dst} mask stationary;
PSUM accumulates numerator and denominator together. All PSUM reads go
through the Scalar engine (DVE PSUM reads are pathologically slow). Layer-2
pre-features are exchanged with one AllGather; the tiny fc + log_softmax head
runs on host.
"""
import os
import sys
import math

import numpy as np
import ml_dtypes


def _setup_paths():
    for p in ("/opt/trn_rl_repo", "/root/.axon_site/_ro/trn_rl_repo"):
        if os.path.isdir(p) and p not in sys.path:
            sys.path.insert(0, p)
    try:
        import concourse.bass  # noqa: F401
    except ImportError as e:
        raise RuntimeError(f"concourse not importable: {e}")


_setup_paths()

import concourse.bass as bass  # noqa: E402
import concourse.mybir as mybir  # noqa: E402
import concourse.tile as tile  # noqa: E402
from concourse import bacc, bass_utils  # noqa: E402

bf16 = ml_dtypes.bfloat16
BF = mybir.dt.bfloat16
F32 = mybir.dt.float32
I16 = mybir.dt.int16
I32 = mybir.dt.int32
AL = mybir.AluOpType
AF = mybir.ActivationFunctionType
AX = mybir.AxisListType


class Cfg:
    def __init__(self, N=50000, E=800000, IN_C=128, HID=64, OUT_C=64, HEADS=4,
                 NCLS=40, NEG=0.2, NCORES=8):
        self.N, self.E = N, E
        self.IN_C, self.HID, self.OUT_C, self.HEADS = IN_C, HID, OUT_C, HEADS
        self.NCLS, self.NEG, self.NCORES = NCLS, NEG, NCORES
        assert N % NCORES == 0
        self.NB = N // NCORES                      # owned real nodes per core
        self.NBLK = math.ceil(self.NB / 128)       # dst blocks per core
        self.NDP = self.NBLK * 128                 # padded owned rows per core
        self.RTOT = self.NDP * NCORES              # global padded row space
        assert self.RTOT % 128 == 0
        self.NT1 = self.RTOT // 128                # phase-A node tiles
        # A/B gather split (int16 row-index limit), multiple of 128
        self.SPLIT = min(32768, (self.RTOT // 2 + 127) // 128 * 128)
        assert self.SPLIT % 128 == 0 and self.SPLIT < 32768 + 1
        self.C1 = HEADS * HID                      # 256 layer-1 channels
        self.ROW1 = 384 if self.C1 == 256 else (
            (self.C1 + 8 + 127) // 128 * 128)      # table1 cols (768B rows)
        self.ROW2 = 128                            # cc3 cols (256B rows)
        assert self.C1 + 8 <= self.ROW1 and self.OUT_C + 2 <= self.ROW2

    def row_of(self, v):
        return self.NDP * (v // self.NB) + (v % self.NB)


def _pack_idx(vals_2d):
    """vals_2d [G, n] -> dma_gather index layout [G, 128, n//16] int16.

    Index i lives at [i % 16, i // 16]; the 16-row group is replicated 8x
    across the 128 partitions.
    """
    G, n = vals_2d.shape
    assert n % 16 == 0
    a = vals_2d.reshape(G, n // 16, 16).transpose(0, 2, 1)   # [G, 16, n/16]
    return np.tile(a, (1, 8, 1)).astype(np.int16)            # [G, 128, n/16]


def host_prep(cfg, x, edge_index, W1, att_src1, att_dst1, b1, W2, att_src2,
              att_dst2, b2):
    """Build per-core in_maps. Self-loops excluded (device applies them from
    resident own-node data); slots sorted by source row; -1 padding so the
    SWDGE only generates descriptors for real edges."""
    c = cfg
    src = np.asarray(edge_index[0], dtype=np.int64)
    dst = np.asarray(edge_index[1], dtype=np.int64)
    EE = src.shape[0]

    core = dst // c.NB
    drel = dst % c.NB
    blk = drel // 128
    din = drel % 128                               # dst index within block
    srow = c.row_of(src)
    isB = (srow >= c.SPLIT).astype(np.int64)

    gid = (core * c.NBLK + blk) * 2 + isB          # group id (A/B separate)
    order = np.argsort(gid, kind="stable")        # stable: random srow order
    gid_s = gid[order]
    counts = np.bincount(gid_s, minlength=c.NCORES * c.NBLK * 2)
    nA = counts[0::2].reshape(c.NCORES, c.NBLK)
    nB = counts[1::2].reshape(c.NCORES, c.NBLK)
    kA = max(1, int(math.ceil(nA.max() / 128)))
    kB = max(1, int(math.ceil(nB.max() / 128)))
    K = kA + kB

    # rank within group
    starts = np.zeros_like(counts)
    starts[1:] = np.cumsum(counts)[:-1]
    rank = np.arange(EE) - starts[gid_s]

    slot = np.where(isB[order] == 0, rank, kA * 128 + rank)
    cg = core[order] * c.NBLK + blk[order]          # [EE] group (core, blk)

    # pad slots hold -1: the SWDGE skips them (no descriptor, no transfer);
    # dstrel=128 masks them out of the aggregation
    srow_slot = np.full((c.NCORES * c.NBLK, K * 128), -1, np.int64)
    din_slot = np.full((c.NCORES * c.NBLK, K * 128), 128.0, np.float32)
    srow_slot[cg, slot] = srow[order]
    din_slot[cg, slot] = din[order]

    srow_slot = srow_slot.reshape(c.NCORES, c.NBLK, K * 128)
    din_slot = din_slot.reshape(c.NCORES, c.NBLK, K * 128)

    idxB_vals = srow_slot[:, :, kA * 128:].copy()
    idxB_vals[idxB_vals >= 0] -= c.SPLIT
    cnts = np.stack([nA, nB], axis=-1).astype(np.int32)   # [NCORES, NBLK, 2]

    # augmented weights
    W1 = np.asarray(W1, np.float32)
    a_s1 = np.asarray(att_src1, np.float32).reshape(c.HEADS, c.HID)
    a_d1 = np.asarray(att_dst1, np.float32).reshape(c.HEADS, c.HID)
    W1r = W1.reshape(c.IN_C, c.HEADS, c.HID)
    Wa_s = np.einsum("khc,hc->kh", W1r, a_s1)       # [IN_C, HEADS]
    Wa_d = np.einsum("khc,hc->kh", W1r, a_d1)
    w1aug = np.zeros((c.IN_C, c.C1 + 8), np.float32)
    w1aug[:, :c.C1] = W1
    w1aug[:, c.C1:c.C1 + c.HEADS] = Wa_s
    w1aug[:, c.C1 + 4:c.C1 + 4 + c.HEADS] = Wa_d

    W2 = np.asarray(W2, np.float32)
    a_s2 = np.asarray(att_src2, np.float32).reshape(c.OUT_C)
    a_d2 = np.asarray(att_dst2, np.float32).reshape(c.OUT_C)
    w2aug = np.zeros((c.C1, 72), np.float32)
    w2aug[:, :c.OUT_C] = W2
    w2aug[:, c.OUT_C] = W2 @ a_s2
    w2aug[:, c.OUT_C + 1] = W2 @ a_d2

    assert np.allclose(np.asarray(b1), 0) and np.allclose(np.asarray(b2), 0), \
        "nonzero biases not folded in this build"

    # padded, row-mapped, transposed x tiles (own rows only - table build is
    # sharded and exchanged with an AllGather)
    x = np.asarray(x, np.float32)
    x_pad = np.zeros((c.RTOT, c.IN_C), np.float32)
    rows = c.row_of(np.arange(c.N))
    x_pad[rows] = x
    xT = x_pad.reshape(c.NT1, 128, c.IN_C).transpose(0, 2, 1)  # [t, k, n]
    xT = np.ascontiguousarray(xT).astype(bf16)

    iota = np.broadcast_to(np.arange(128, dtype=np.float32),
                           (128, 128)).astype(bf16).copy()
    eye = np.eye(128, dtype=np.float32).astype(bf16)
    ones1 = np.ones((1, 128), np.float32).astype(bf16)
    ones = np.ones((128, 1), np.float32)
    # e4[q, h*128+p] = (q == h): per-head row-selector for the rank-1 replicate
    e4 = np.zeros((cfg.HEADS, cfg.HEADS * 128), np.float32)
    for h in range(cfg.HEADS):
        e4[h, h * 128:(h + 1) * 128] = 1.0
    e4 = e4.astype(bf16)

    in_maps = []
    meta = dict(kA=kA, kB=kB, K=K)
    for ci in range(c.NCORES):
        idxA = _pack_idx(srow_slot[ci, :, :kA * 128])             # [NBLK,128,kA*8]
        idxB = _pack_idx(idxB_vals[ci])
        dr = din_slot[ci].reshape(c.NBLK, K, 128).transpose(2, 0, 1)  # [128,NBLK,K]
        xo = xT[ci * c.NBLK:(ci + 1) * c.NBLK]                    # own tiles
        in_maps.append({
            "x_own": np.ascontiguousarray(xo),
            "w1aug": w1aug.astype(bf16),
            "w2aug": np.ascontiguousarray(
                w2aug.astype(bf16).reshape(c.C1 // 128, 128, 72).transpose(1, 0, 2)),
            "idxA": np.ascontiguousarray(idxA.transpose(1, 0, 2)),  # [128,NBLK,kA*8]
            "idxB": np.ascontiguousarray(idxB.transpose(1, 0, 2)),
            "cnts": cnts[ci].reshape(1, c.NBLK, 2),
            "dstrel": np.ascontiguousarray(dr).astype(bf16),
            "iota": iota,
            "eye": eye,
            "ones1": ones1,
            "ones": ones,
            "e4": e4,
        })
    return in_maps, meta


def build(cfg, kA, kB, debug=False, stage="F"):
    """stage: truncate program after phase A/B/C/D/E/F (for HW bisection)."""
    c = cfg
    K = kA + kB
    KH = c.C1 // 128                      # k-halves for layer-2 contraction
    H = c.HEADS
    nc = bacc.Bacc("TRN2", target_bir_lowering=False, debug=False,
                   num_devices=c.NCORES)

    # ---- IO ----
    x_own_d = nc.dram_tensor("x_own", [c.NBLK, 128, c.IN_C], BF, kind="ExternalInput").ap()
    w1_d = nc.dram_tensor("w1aug", [c.IN_C, c.C1 + 8], BF, kind="ExternalInput").ap()
    w2_d = nc.dram_tensor("w2aug", [128, KH, 72], BF, kind="ExternalInput").ap()
    idxA_d = nc.dram_tensor("idxA", [128, c.NBLK, kA * 8], I16, kind="ExternalInput").ap()
    idxB_d = nc.dram_tensor("idxB", [128, c.NBLK, kB * 8], I16, kind="ExternalInput").ap()
    cnts_d = nc.dram_tensor("cnts", [1, c.NBLK, 2], I32, kind="ExternalInput").ap()
    dstrel_d = nc.dram_tensor("dstrel", [128, c.NBLK, K], BF, kind="ExternalInput").ap()
    iota_d = nc.dram_tensor("iota", [128, 128], BF, kind="ExternalInput").ap()
    eye_d = nc.dram_tensor("eye", [128, 128], BF, kind="ExternalInput").ap()
    ones1_d = nc.dram_tensor("ones1", [1, 128], BF, kind="ExternalInput").ap()
    ones_d = nc.dram_tensor("ones", [128, 1], F32, kind="ExternalInput").ap()
    e4_d = nc.dram_tensor("e4", [H, H * 128], BF, kind="ExternalInput").ap()
    pool_d = nc.dram_tensor("pool64", [c.OUT_C, 1], F32, kind="ExternalOutput").ap()
    if debug:
        h1dbg_d = nc.dram_tensor("h1dbg", [c.NDP, c.C1], F32, kind="ExternalOutput").ap()
        h2dbg_d = nc.dram_tensor("h2dbg", [c.NDP, 72], F32, kind="ExternalOutput").ap()

    # ---- internal DRAM ----
    t1own = nc.dram_tensor("t1own", [c.NDP, c.ROW1], BF, kind="Internal").ap()
    t1full = nc.dram_tensor("t1full", [c.RTOT, c.ROW1], BF, kind="Internal",
                            addr_space="Shared").ap()
    cc3in = nc.dram_tensor("cc3in", [c.NDP, c.ROW2], BF, kind="Internal").ap()
    cc3 = nc.dram_tensor("cc3", [c.RTOT, c.ROW2], BF, kind="Internal",
                         addr_space="Shared").ap()
    t3B = nc.dram_tensor("t3B", [c.RTOT - c.SPLIT, c.ROW2], BF, kind="Internal").ap()

    with tile.TileContext(nc) as tc:
        with tc.tile_pool(name="const", bufs=1) as cpool, \
             tc.tile_pool(name="pa", bufs=3) as pa, \
             tc.tile_pool(name="ppA", bufs=2, space="PSUM") as ppA, \
             tc.tile_pool(name="ppB", bufs=2, space="PSUM") as ppB, \
             tc.tile_pool(name="ppS", bufs=1, space="PSUM") as ppS, \
             tc.tile_pool(name="pgh", bufs=3) as pgh, \
             tc.tile_pool(name="pg", bufs=2) as pg, \
             tc.tile_pool(name="pt", bufs=1) as pt, \
             tc.tile_pool(name="pe2", bufs=2) as pe2, \
             tc.tile_pool(name="sm", bufs=3) as sm:

            # constants resident
            w1s = cpool.tile_from(w1_d)                     # [128, C1+8]
            w2s = cpool.tile_from(w2_d)                     # [128, KH, 72]
            iota_s = cpool.tile_from(iota_d)
            eye_s = cpool.tile_from(eye_d)
            ones1_s = cpool.tile_from(ones1_d)
            ones_s = cpool.tile_from(ones_d)
            e4_s = cpool.tile_from(e4_d)
            idxA_s = cpool.tile_from(idxA_d)
            idxB_s = cpool.tile_from(idxB_d)
            cnts_s = cpool.tile_from(cnts_d)
            dstrel_s = cpool.tile_from(dstrel_d)

            hown_s = cpool.tile([128, c.NBLK, c.C1 + 8], BF)   # own h + alphas
            eeself_s = cpool.tile([128, c.NBLK, H], F32)
            c3own_s = cpool.tile([128, c.NBLK, 72], BF)        # own h2pre+alphas
            eeself2_s = cpool.tile([128, c.NBLK, 1], F32)
            pacc = cpool.tile([128, c.OUT_C], F32)
            nc.vector.memset(pacc[:], 0.0)

            # gather-count registers (gpsimd in-order execution makes reuse
            # across blocks safe)
            regA = nc.gpsimd.alloc_register("cntA")
            regB = nc.gpsimd.alloc_register("cntB")

            # ========= phase A: own-shard h tiles, then AllGather table ======
            for j in range(c.NBLK):
                xo = pa.tile([128, c.IN_C], BF, tag="xo")
                nc.sync.dma_start(out=xo[:], in_=x_own_d[j, :, :])
                pso = ppA.tile([128, c.C1 + 8], F32, tag="A")
                nc.tensor.matmul(out=pso[:], lhsT=xo[:], rhs=w1s[:],
                                 start=True, stop=True)
                nc.scalar.activation(out=hown_s[:, j, :], in_=pso[:], func=AF.Copy)
                nc.sync.dma_start(
                    out=t1own[j * 128:(j + 1) * 128, 0:c.C1 + 8],
                    in_=hown_s[:, j, :])

            nc.gpsimd.collective_compute(
                kind="AllGather", op=AL.bypass,
                replica_groups=[list(range(c.NCORES))],
                ins=[t1own[:, :]], outs=[t1full[:, :]])

            # self-loop weights for all blocks: exp(lrelu(as_own + ad_own))
            zs = sm.tile([128, c.NBLK, H], F32, tag="zs")
            nc.vector.tensor_tensor(
                out=zs[:], in0=hown_s[:, :, c.C1:c.C1 + H],
                in1=hown_s[:, :, c.C1 + 4:c.C1 + 4 + H], op=AL.add)
            lrs = sm.tile([128, c.NBLK, H], F32, tag="lrs")
            nc.vector.scalar_tensor_tensor(
                out=lrs[:], in0=zs[:], scalar=c.NEG, in1=zs[:],
                op0=AL.mult, op1=AL.max)
            nc.scalar.activation(out=eeself_s[:], in_=lrs[:], func=AF.Exp)

            # ===== phase B+C interleaved: layer-1 aggregation + h2_pre =======
            if stage >= "B":
                for i in range(3):   # pre-zero rotating hc pad cols + hg bufs
                    hcz = pa.tile([128, c.ROW2], BF, tag="hc")
                    nc.vector.memset(hcz[:], 0.0)
                    hgz = pgh.tile([128, K, c.ROW1], BF, tag="hg")
                    nc.vector.memset(hgz[:], 0.0)
                for i in range(2):
                    hgz2 = pe2.tile([128, K, c.ROW2], BF, tag="hg2")
                    nc.vector.memset(hgz2[:], 0.0)
            for b in (range(c.NBLK) if stage >= "B" else []):
                hg = pgh.tile([128, K, c.ROW1], BF, tag="hg")
                nc.gpsimd.reg_load(regA, cnts_s[0:1, b, 0:1])
                nc.gpsimd.reg_load(regB, cnts_s[0:1, b, 1:2])
                nc.gpsimd.dma_gather(
                    out_ap=hg[:, 0:kA, :], in_ap=t1full[0:c.SPLIT, :],
                    idxs_ap=idxA_s[:, b, :], num_idxs=kA * 128,
                    num_idxs_reg=regA, elem_size=c.ROW1, single_packet=False)
                nc.gpsimd.dma_gather(
                    out_ap=hg[:, kA:K, :], in_ap=t1full[c.SPLIT:c.RTOT, :],
                    idxs_ap=idxB_s[:, b, :], num_idxs=kB * 128,
                    num_idxs_reg=regB, elem_size=c.ROW1, single_packet=False)

                mask = pg.tile([128, K, 128], BF, tag="mask")
                nc.vector.tensor_tensor(
                    out=mask[:],
                    in0=iota_s[:, None, :].to_broadcast([128, K, 128]),
                    in1=dstrel_s[:, b, :, None].to_broadcast([128, K, 128]),
                    op=AL.is_equal)

                # dst-alpha broadcast: adT = own_ad^T, rank-1 replicate per head
                adTf = ppS.tile([128, 128], BF, tag="adT")
                adTp = adTf[0:H, :]
                nc.tensor.transpose(
                    out=adTp[:], in_=hown_s[:, b, c.C1 + 4:c.C1 + 4 + H],
                    identity=eye_s[:])
                adT = sm.tile([H, 128], BF, tag="adTs")
                nc.scalar.activation(out=adT[:], in_=adTp[:], func=AF.Copy)
                adrp = ppS.tile([128, H, 128], F32, tag="adrp")
                for h in range(H):
                    nc.tensor.matmul(out=adrp[:, h, :],
                                     lhsT=e4_s[:, h * 128:(h + 1) * 128],
                                     rhs=adT[:], start=True, stop=True)
                # head-pair-packed replicate: two bf16 ad values live in one
                # fp32 carrier; 1.0*x and 0+x are exact, so the one-hot
                # select preserves bits (carrier exponent byte is the bf16
                # exponent -> always a normal fp32).
                adrepP = sm.tile([128, 256], F32, tag="adrep")
                bfv = adrepP[:].bitcast(BF)
                nc.scalar.activation(
                    out=bfv.rearrange("p (c d w) -> p c w d", c=2, d=128, w=2),
                    in_=adrp[:].rearrange("p (c w) d -> p c w d", c=2, w=2),
                    func=AF.Copy)

                # per-edge dst alpha: zad[p,j,h] = sum_d mask[p,j,d]*adrep[p,h,d]
                tselP = pt.tile([128, K, 2, 128], F32, tag="tsel")
                nc.vector.tensor_tensor(
                    out=tselP[:],
                    in0=mask[:, :, None, :].to_broadcast([128, K, 2, 128]),
                    in1=adrepP[:].rearrange("p (c d) -> p c d", c=2)[
                        :, None, :, :].to_broadcast([128, K, 2, 128]),
                    op=AL.mult)
                zadP = sm.tile([128, K, 2], F32, tag="zad")
                nc.vector.tensor_reduce(out=zadP[:], in_=tselP[:], axis=AX.X,
                                        op=AL.add)

                z = sm.tile([128, K, H], F32, tag="z")
                nc.vector.tensor_tensor(
                    out=z[:], in0=hg[:, :, c.C1:c.C1 + H],
                    in1=zadP[:].bitcast(BF), op=AL.add)
                lr = sm.tile([128, K, H], F32, tag="lr")
                nc.vector.scalar_tensor_tensor(
                    out=lr[:], in0=z[:], scalar=c.NEG, in1=z[:],
                    op0=AL.mult, op1=AL.max)
                eeb = sm.tile([128, K, H], BF, tag="eeb")
                nc.scalar.activation(out=eeb[:], in_=lr[:], func=AF.Exp)

                v = pg.tile([128, K, c.C1 + 4], BF, tag="v")
                nc.vector.tensor_tensor(
                    out=v[:, :, 0:c.C1].rearrange("p k (h q) -> p k h q", h=H),
                    in0=hg[:, :, 0:c.C1].rearrange("p k (h q) -> p k h q", h=H),
                    in1=eeb[:, :, :, None].to_broadcast([128, K, H, c.HID]),
                    op=AL.mult)
                nc.scalar.activation(out=v[:, :, c.C1:c.C1 + 4], in_=eeb[:],
                                     func=AF.Copy)

                # self-loop row: vself[p,:] = eeself[p]*[h_own | 1] folded into
                # the aggregation as an extra identity-mask matmul
                vself = sm.tile([128, c.C1 + 4], BF, tag="vself")
                for h in range(H):
                    nc.scalar.activation(
                        out=vself[:, h * c.HID:(h + 1) * c.HID],
                        in_=hown_s[:, b, h * c.HID:(h + 1) * c.HID],
                        func=AF.Copy, scale=eeself_s[:, b, h:h + 1])
                nc.scalar.activation(out=vself[:, c.C1:c.C1 + 4],
                                     in_=eeself_s[:, b, :], func=AF.Copy)

                ps = ppB.tile([128, c.C1 + 4], F32, tag="B")
                for j in range(K):
                    nc.tensor.matmul(out=ps[:], lhsT=mask[:, j, :],
                                     rhs=v[:, j, :],
                                     start=(j == 0), stop=False)
                nc.tensor.matmul(out=ps[:], lhsT=eye_s[:], rhs=vself[:],
                                 start=False, stop=True)

                sbn = sm.tile([128, c.C1 + 4], F32, tag="sbn")
                nc.scalar.activation(out=sbn[:], in_=ps[:], func=AF.Copy)
                rec = sm.tile([128, H], F32, tag="rec")
                nc.vector.reciprocal(out=rec[:], in_=sbn[:, c.C1:c.C1 + 4])
                h1b = sm.tile([128, c.C1], BF, tag="h1b")
                nc.vector.scalar_tensor_tensor(
                    out=h1b[:].rearrange("p (h q) -> p h q", h=H),
                    in0=sbn[:, 0:c.C1].rearrange("p (h q) -> p h q", h=H),
                    scalar=0.0, op0=AL.max,
                    in1=rec[:, :, None].to_broadcast([128, H, c.HID]),
                    op1=AL.mult)
                if debug:
                    h1f = sm.tile([128, c.C1], F32, tag="h1f")
                    nc.vector.scalar_tensor_tensor(
                        out=h1f[:].rearrange("p (h q) -> p h q", h=H),
                        in0=sbn[:, 0:c.C1].rearrange("p (h q) -> p h q", h=H),
                        scalar=0.0, op0=AL.max,
                        in1=rec[:, :, None].to_broadcast([128, H, c.HID]),
                        op1=AL.mult)
                    nc.sync.dma_start(out=h1dbg_d[b * 128:(b + 1) * 128, :],
                                      in_=h1f[:])

                # ---- phase C for this block: h2_pre = h1 @ W2aug ----
                if stage >= "C":
                    pscf = ppA.tile([128, c.C1 + 8], F32, tag="A")
                    psc = pscf[:, 0:72]
                    for kh in range(KH):
                        htf = ppS.tile([128, 128], BF, tag="adT")
                        nc.tensor.transpose(
                            out=htf[:], in_=h1b[:, kh * 128:(kh + 1) * 128],
                            identity=eye_s[:])
                        ht = pa.tile([128, 128], BF, tag="ht")
                        nc.scalar.activation(out=ht[:], in_=htf[:], func=AF.Copy)
                        nc.tensor.matmul(out=psc[:], lhsT=ht[:], rhs=w2s[:, kh, :],
                                         start=(kh == 0), stop=(kh == KH - 1))
                    hc = pa.tile([128, c.ROW2], BF, tag="hc")
                    nc.scalar.activation(out=hc[:, 0:72], in_=psc[:], func=AF.Copy)
                    nc.scalar.activation(out=c3own_s[:, b, :], in_=hc[:, 0:72],
                                         func=AF.Copy)
                    nc.sync.dma_start(out=cc3in[b * 128:(b + 1) * 128, :], in_=hc[:])
                    if debug:
                        h2f = pa.tile([128, 72], F32, tag="h2f")
                        nc.vector.tensor_copy(out=h2f[:], in_=psc[:])
                        nc.sync.dma_start(out=h2dbg_d[b * 128:(b + 1) * 128, :],
                                          in_=h2f[:])

            if stage >= "C":
                zs2 = sm.tile([128, c.NBLK, 1], F32, tag="zs2")
                nc.vector.tensor_tensor(
                    out=zs2[:], in0=c3own_s[:, :, c.OUT_C:c.OUT_C + 1],
                    in1=c3own_s[:, :, c.OUT_C + 1:c.OUT_C + 2], op=AL.add)
                lrs2 = sm.tile([128, c.NBLK, 1], F32, tag="lrs2")
                nc.vector.scalar_tensor_tensor(
                    out=lrs2[:], in0=zs2[:], scalar=c.NEG, in1=zs2[:],
                    op0=AL.mult, op1=AL.max)
                nc.scalar.activation(out=eeself2_s[:], in_=lrs2[:], func=AF.Exp)

            # ================= phase D: allgather + repack ===================
            if stage >= "D":
                nc.gpsimd.collective_compute(
                    kind="AllGather", op=AL.bypass,
                    replica_groups=[list(range(c.NCORES))],
                    ins=[cc3in[:, :]], outs=[cc3[:, :]])
                nc.sync.dma_start(out=t3B[:, :], in_=cc3[c.SPLIT:c.RTOT, :])

            # ================= phase E: layer-2 edge aggregation =============
            for b in (range(c.NBLK) if stage >= "E" else []):
                hg2 = pe2.tile([128, K, c.ROW2], BF, tag="hg2")
                nc.gpsimd.reg_load(regA, cnts_s[0:1, b, 0:1])
                nc.gpsimd.reg_load(regB, cnts_s[0:1, b, 1:2])
                nc.gpsimd.dma_gather(
                    out_ap=hg2[:, 0:kA, :], in_ap=cc3[:, :],
                    idxs_ap=idxA_s[:, b, :], num_idxs=kA * 128,
                    num_idxs_reg=regA, elem_size=c.ROW2, single_packet=False)
                nc.gpsimd.dma_gather(
                    out_ap=hg2[:, kA:K, :], in_ap=t3B[:, :],
                    idxs_ap=idxB_s[:, b, :], num_idxs=kB * 128,
                    num_idxs_reg=regB, elem_size=c.ROW2, single_packet=False)

                mask2 = pe2.tile([128, K, 128], BF, tag="mask2")
                nc.vector.tensor_tensor(
                    out=mask2[:],
                    in0=iota_s[:, None, :].to_broadcast([128, K, 128]),
                    in1=dstrel_s[:, b, :, None].to_broadcast([128, K, 128]),
                    op=AL.is_equal)

                adTp2 = ppS.tile([1, 128], BF, tag="adT2")
                nc.tensor.transpose(
                    out=adTp2[:], in_=c3own_s[:, b, c.OUT_C + 1:c.OUT_C + 2],
                    identity=eye_s[:])
                adT2 = sm.tile([1, 128], BF, tag="adT2s")
                nc.scalar.activation(out=adT2[:], in_=adTp2[:], func=AF.Copy)
                adrp2 = ppS.tile([128, 128], F32, tag="adrp2")
                nc.tensor.matmul(out=adrp2[:], lhsT=ones1_s[:], rhs=adT2[:],
                                 start=True, stop=True)
                adrep2 = sm.tile([128, 128], BF, tag="adrep2")
                nc.scalar.activation(out=adrep2[:], in_=adrp2[:], func=AF.Copy)

                tsel2 = pt.tile([128, K, 128], BF, tag="tsel2")
                nc.vector.tensor_tensor(
                    out=tsel2[:], in0=mask2[:],
                    in1=adrep2[:, None, :].to_broadcast([128, K, 128]),
                    op=AL.mult)
                zad2 = sm.tile([128, K, 1], BF, tag="zad2")
                with nc.allow_low_precision("one-hot select, sum is exact"):
                    nc.vector.tensor_reduce(out=zad2[:], in_=tsel2[:], axis=AX.X,
                                            op=AL.add)

                z2 = sm.tile([128, K, 1], F32, tag="z2")
                nc.vector.tensor_tensor(
                    out=z2[:], in0=hg2[:, :, c.OUT_C:c.OUT_C + 1], in1=zad2[:],
                    op=AL.add)
                lr2 = sm.tile([128, K, 1], F32, tag="lr2")
                nc.vector.scalar_tensor_tensor(
                    out=lr2[:], in0=z2[:], scalar=c.NEG, in1=z2[:],
                    op0=AL.mult, op1=AL.max)
                ee2 = sm.tile([128, K, 1], BF, tag="ee2")
                nc.scalar.activation(out=ee2[:], in_=lr2[:], func=AF.Exp)

                v2 = pe2.tile([128, K, c.OUT_C + 1], BF, tag="v2")
                nc.vector.tensor_tensor(
                    out=v2[:, :, 0:c.OUT_C], in0=hg2[:, :, 0:c.OUT_C],
                    in1=ee2[:, :, 0:1].to_broadcast([128, K, c.OUT_C]),
                    op=AL.mult)
                nc.scalar.activation(out=v2[:, :, c.OUT_C:c.OUT_C + 1],
                                     in_=ee2[:], func=AF.Copy)

                vself2 = sm.tile([128, c.OUT_C + 1], BF, tag="vself2")
                nc.scalar.activation(
                    out=vself2[:, 0:c.OUT_C], in_=c3own_s[:, b, 0:c.OUT_C],
                    func=AF.Copy, scale=eeself2_s[:, b, 0:1])
                nc.scalar.activation(out=vself2[:, c.OUT_C:c.OUT_C + 1],
                                     in_=eeself2_s[:, b, :], func=AF.Copy)

                ps2f = ppB.tile([128, c.C1 + 4], F32, tag="B")
                ps2 = ps2f[:, 0:c.OUT_C + 1]
                for j in range(K):
                    nc.tensor.matmul(out=ps2[:], lhsT=mask2[:, j, :],
                                     rhs=v2[:, j, :],
                                     start=(j == 0), stop=False)
                nc.tensor.matmul(out=ps2[:], lhsT=eye_s[:], rhs=vself2[:],
                                 start=False, stop=True)

                sbn2 = sm.tile([128, c.OUT_C + 1], F32, tag="sbn2")
                nc.scalar.activation(out=sbn2[:], in_=ps2[:], func=AF.Copy)
                rec2 = sm.tile([128, 1], F32, tag="rec2")
                nc.vector.reciprocal(out=rec2[:], in_=sbn2[:, c.OUT_C:c.OUT_C + 1])
                o2 = sm.tile([128, c.OUT_C], F32, tag="o2")
                nc.vector.scalar_tensor_tensor(
                    out=o2[:], in0=sbn2[:, 0:c.OUT_C], scalar=0.0, op0=AL.max,
                    in1=rec2[:, 0:1].to_broadcast([128, c.OUT_C]), op1=AL.mult)
                nc.vector.tensor_tensor(out=pacc[:], in0=pacc[:], in1=o2[:],
                                        op=AL.add)

            # ================= phase F: pool partial =========================
            psff = ppB.tile([128, c.C1 + 4], F32, tag="B")
            psf = psff[0:c.OUT_C, 0:1]
            nc.tensor.matmul(out=psf[:], lhsT=pacc[:], rhs=ones_s[:],
                             start=True, stop=True)
            pf = sm.tile([c.OUT_C, 1], F32, tag="pf")
            nc.scalar.activation(out=pf[:], in_=psf[:], func=AF.Copy)
            nc.sync.dma_start(out=pool_d[:, :], in_=pf[:])

    nc.compile()
    legalize_waits(nc)
    return nc


def legalize_waits(nc):
    """Walrus encodes at most ONE sync wait per instruction on this toolchain.
    Hoist excess waits onto same-engine NoOps inserted before the instruction."""
    for fn in nc.m.functions:
        for bb in fn.blocks:
            insts = list(bb.instructions)
            out = []
            changed = False
            for inst in insts:
                si = inst.sync_info
                if si is not None and si.on_wait and len(si.on_wait) > 1:
                    waits = list(si.on_wait)
                    for w in waits[:-1]:
                        nop = mybir.InstNoOp(
                            name=nc.get_next_instruction_name(), ins=[], outs=[])
                        nop.engine = inst.engine
                        nop.sync_info = mybir.SyncInfo(on_wait=[w], on_update=[])
                        nc.register_instruction(nop)
                        out.append(nop)
                    inst.sync_info = mybir.SyncInfo(
                        on_wait=waits[-1:], on_update=list(si.on_update))
                    changed = True
                out.append(inst)
            if changed:
                bb.instructions.clear()
                bb.instructions.extend(out)


def host_finish(cfg, pools, fc_w, fc_b):
    c = cfg
    tot = np.zeros(c.OUT_C, np.float64)
    for p in pools:
        tot += p[:, 0].astype(np.float64)
    pooled = (tot / c.N).astype(np.float32)
    logits = pooled @ np.asarray(fc_w, np.float32) + np.asarray(fc_b, np.float32)
    m = logits.max()
    ls = logits - (m + np.log(np.exp(logits - m).sum()))
    return ls.reshape(1, c.NCLS).astype(np.float32)


_BUILD_CACHE = {}


def run(cfg, inputs, debug=False, trace=False, **run_kwargs):
    in_maps, meta = host_prep(
        cfg, inputs["x"], inputs["edge_index"], inputs["W1"], inputs["att_src1"],
        inputs["att_dst1"], inputs["b1"], inputs["W2"], inputs["att_src2"],
        inputs["att_dst2"], inputs["b2"])
    stage = os.environ.get("KSTAGE", "F")
    key = (cfg.N, cfg.E, meta["kA"], meta["kB"], debug, stage)
    if key not in _BUILD_CACHE:
        _BUILD_CACHE[key] = build(cfg, meta["kA"], meta["kB"], debug=debug,
                                  stage=stage)
    nc = _BUILD_CACHE[key]
    res = bass_utils.run_bass_kernel_spmd(
        nc, in_maps, core_ids=list(range(cfg.NCORES)), trace=trace, **run_kwargs)
    out = host_finish(cfg, [r["pool64"] for r in res.results],
                      inputs["fc_w"], inputs["fc_b"])
    return out, res


def kernel(**inputs):
    cfg = Cfg()
    out, _ = run(cfg, inputs)
    return out


# revision 28
# speedup vs baseline: 1.0310x; 1.0310x over previous
"""GAT (2-layer graph attention network) Bass kernel for 8 Trainium2 NeuronCores.

Strategy: edges partitioned by destination-node block (N/8 dst nodes per core,
blocks of 128). Layer-1 node features (h = x @ W1aug, alphas fused via an
augmented weight matrix) are computed replicated on every core into a DRAM
table sharded 8-way and exchanged with an AllGather; per-edge source rows
arrive via SWDGE gather DMAs (slot order deliberately NOT sorted by source
row - sorted indices serialize the 16 round-robin SDMA engines onto the same
DRAM banks). Self-loop edges never enter the gathers - they are applied from
SBUF-resident own-node tiles as an extra identity-mask matmul. Dst-side
alphas are never gathered either: they are broadcast across partitions with a
PE transpose + rank-1 selector matmuls (two bf16 values packed per fp32
carrier - 1.0*x / 0+x are exact, so the one-hot select preserves bits) and
selected per edge with a masked DVE multiply+reduce against the same one-hot
mask used for aggregation. Segment softmax + weighted aggregation run as PE
matmuls with the one-hot {slot x dst} mask stationary; PSUM accumulates
numerator and denominator together. h1 -> h2_pre happens on-chip via PE
transposes (no DRAM round-trip, and DMA-transpose would serialize against
gather DMAs). All PSUM reads go through the Scalar engine (DVE PSUM reads are
pathologically slow); scalar activations stay within {Copy, Exp} to avoid
ACT-table reloads. Layer-2 pre-features are exchanged with a second
AllGather; the tiny fc + log_softmax head runs on host.
"""
import os
import sys
import math

import numpy as np
import ml_dtypes


def _setup_paths():
    for p in ("/opt/trn_rl_repo", "/root/.axon_site/_ro/trn_rl_repo"):
        if os.path.isdir(p) and p not in sys.path:
            sys.path.insert(0, p)
    try:
        import concourse.bass  # noqa: F401
    except ImportError as e:
        raise RuntimeError(f"concourse not importable: {e}")


_setup_paths()

import concourse.bass as bass  # noqa: E402
import concourse.mybir as mybir  # noqa: E402
import concourse.tile as tile  # noqa: E402
from concourse import bacc, bass_utils  # noqa: E402

bf16 = ml_dtypes.bfloat16
BF = mybir.dt.bfloat16
F32 = mybir.dt.float32
I16 = mybir.dt.int16
I32 = mybir.dt.int32
AL = mybir.AluOpType
AF = mybir.ActivationFunctionType
AX = mybir.AxisListType


class Cfg:
    def __init__(self, N=50000, E=800000, IN_C=128, HID=64, OUT_C=64, HEADS=4,
                 NCLS=40, NEG=0.2, NCORES=8):
        self.N, self.E = N, E
        self.IN_C, self.HID, self.OUT_C, self.HEADS = IN_C, HID, OUT_C, HEADS
        self.NCLS, self.NEG, self.NCORES = NCLS, NEG, NCORES
        assert N % NCORES == 0
        self.NB = N // NCORES                      # owned real nodes per core
        self.NBLK = math.ceil(self.NB / 128)       # dst blocks per core
        self.NDP = self.NBLK * 128                 # padded owned rows per core
        self.RTOT = self.NDP * NCORES              # global padded row space
        assert self.RTOT % 128 == 0
        self.NT1 = self.RTOT // 128                # phase-A node tiles
        # A/B gather split (int16 row-index limit), multiple of 128
        self.SPLIT = min(32768, (self.RTOT // 2 + 127) // 128 * 128)
        assert self.SPLIT % 128 == 0 and self.SPLIT < 32768 + 1
        self.C1 = HEADS * HID                      # 256 layer-1 channels
        self.ROW1 = 384 if self.C1 == 256 else (
            (self.C1 + 8 + 127) // 128 * 128)      # table1 cols (768B rows)
        self.ROW2 = 128                            # cc3 cols (256B rows)
        assert self.C1 + 8 <= self.ROW1 and self.OUT_C + 2 <= self.ROW2

    def row_of(self, v):
        return self.NDP * (v // self.NB) + (v % self.NB)


def _pack_idx(vals_2d):
    """vals_2d [G, n] -> dma_gather index layout [G, 128, n//16] int16.

    Index i lives at [i % 16, i // 16]; the 16-row group is replicated 8x
    across the 128 partitions.
    """
    G, n = vals_2d.shape
    assert n % 16 == 0
    a = vals_2d.reshape(G, n // 16, 16).transpose(0, 2, 1)   # [G, 16, n/16]
    return np.tile(a, (1, 8, 1)).astype(np.int16)            # [G, 128, n/16]


def host_prep(cfg, x, edge_index, W1, att_src1, att_dst1, b1, W2, att_src2,
              att_dst2, b2):
    """Build per-core in_maps. Self-loops excluded (device applies them from
    resident own-node data); slots sorted by source row; -1 padding so the
    SWDGE only generates descriptors for real edges."""
    c = cfg
    src = np.asarray(edge_index[0], dtype=np.int64)
    dst = np.asarray(edge_index[1], dtype=np.int64)
    EE = src.shape[0]

    core = dst // c.NB
    drel = dst % c.NB
    blk = drel // 128
    din = drel % 128                               # dst index within block
    srow = c.row_of(src)
    isB = (srow >= c.SPLIT).astype(np.int64)

    gid = (core * c.NBLK + blk) * 2 + isB          # group id (A/B separate)
    order = np.argsort(gid, kind="stable")        # stable: random srow order
    gid_s = gid[order]
    counts = np.bincount(gid_s, minlength=c.NCORES * c.NBLK * 2)
    nA = counts[0::2].reshape(c.NCORES, c.NBLK)
    nB = counts[1::2].reshape(c.NCORES, c.NBLK)
    kA = max(1, int(math.ceil(nA.max() / 128)))
    kB = max(1, int(math.ceil(nB.max() / 128)))
    K = kA + kB

    # rank within group
    starts = np.zeros_like(counts)
    starts[1:] = np.cumsum(counts)[:-1]
    rank = np.arange(EE) - starts[gid_s]

    slot = np.where(isB[order] == 0, rank, kA * 128 + rank)
    cg = core[order] * c.NBLK + blk[order]          # [EE] group (core, blk)

    # pad slots gather row 0 (valid data; dstrel=128 masks them out)
    srow_slot = np.zeros((c.NCORES * c.NBLK, K * 128), np.int64)
    srow_slot[:, kA * 128:] = c.SPLIT
    din_slot = np.full((c.NCORES * c.NBLK, K * 128), 128.0, np.float32)
    srow_slot[cg, slot] = srow[order]
    din_slot[cg, slot] = din[order]

    srow_slot = srow_slot.reshape(c.NCORES, c.NBLK, K * 128)
    din_slot = din_slot.reshape(c.NCORES, c.NBLK, K * 128)

    idxB_vals = srow_slot[:, :, kA * 128:] - c.SPLIT

    # augmented weights
    W1 = np.asarray(W1, np.float32)
    a_s1 = np.asarray(att_src1, np.float32).reshape(c.HEADS, c.HID)
    a_d1 = np.asarray(att_dst1, np.float32).reshape(c.HEADS, c.HID)
    W1r = W1.reshape(c.IN_C, c.HEADS, c.HID)
    Wa_s = np.einsum("khc,hc->kh", W1r, a_s1)       # [IN_C, HEADS]
    Wa_d = np.einsum("khc,hc->kh", W1r, a_d1)
    w1aug = np.zeros((c.IN_C, c.C1 + 8), np.float32)
    w1aug[:, :c.C1] = W1
    w1aug[:, c.C1:c.C1 + c.HEADS] = Wa_s
    w1aug[:, c.C1 + 4:c.C1 + 4 + c.HEADS] = Wa_d

    W2 = np.asarray(W2, np.float32)
    a_s2 = np.asarray(att_src2, np.float32).reshape(c.OUT_C)
    a_d2 = np.asarray(att_dst2, np.float32).reshape(c.OUT_C)
    w2aug = np.zeros((c.C1, 72), np.float32)
    w2aug[:, :c.OUT_C] = W2
    w2aug[:, c.OUT_C] = W2 @ a_s2
    w2aug[:, c.OUT_C + 1] = W2 @ a_d2

    assert np.allclose(np.asarray(b1), 0) and np.allclose(np.asarray(b2), 0), \
        "nonzero biases not folded in this build"

    # padded, row-mapped, transposed x tiles (own rows only - table build is
    # sharded and exchanged with an AllGather)
    x = np.asarray(x, np.float32)
    x_pad = np.zeros((c.RTOT, c.IN_C), np.float32)
    rows = c.row_of(np.arange(c.N))
    x_pad[rows] = x
    xT = x_pad.reshape(c.NT1, 128, c.IN_C).transpose(0, 2, 1)  # [t, k, n]
    xT = np.ascontiguousarray(xT).astype(bf16)

    iota = np.broadcast_to(np.arange(128, dtype=np.float32),
                           (128, 128)).astype(bf16).copy()
    eye = np.eye(128, dtype=np.float32).astype(bf16)
    ones1 = np.ones((1, 128), np.float32).astype(bf16)
    ones = np.ones((128, 1), np.float32)
    # e4[q, h*128+p] = (q == h): per-head row-selector for the rank-1 replicate
    e4 = np.zeros((cfg.HEADS, cfg.HEADS * 128), np.float32)
    for h in range(cfg.HEADS):
        e4[h, h * 128:(h + 1) * 128] = 1.0
    e4 = e4.astype(bf16)

    in_maps = []
    meta = dict(kA=kA, kB=kB, K=K)
    for ci in range(c.NCORES):
        idxA = _pack_idx(srow_slot[ci, :, :kA * 128])             # [NBLK,128,kA*8]
        idxB = _pack_idx(idxB_vals[ci])
        dr = din_slot[ci].reshape(c.NBLK, K, 128).transpose(2, 0, 1)  # [128,NBLK,K]
        xo = xT[ci * c.NBLK:(ci + 1) * c.NBLK]                    # own tiles
        in_maps.append({
            "x_own": np.ascontiguousarray(xo),
            "w1aug": w1aug.astype(bf16),
            "w2aug": np.ascontiguousarray(
                w2aug.astype(bf16).reshape(c.C1 // 128, 128, 72).transpose(1, 0, 2)),
            "idxA": np.ascontiguousarray(idxA.transpose(1, 0, 2)),  # [128,NBLK,kA*8]
            "idxB": np.ascontiguousarray(idxB.transpose(1, 0, 2)),
            "dstrel": np.ascontiguousarray(dr).astype(bf16),
            "iota": iota,
            "eye": eye,
            "ones1": ones1,
            "ones": ones,
            "e4": e4,
        })
    return in_maps, meta


def build(cfg, kA, kB, debug=False, stage="F"):
    """stage: truncate program after phase A/B/C/D/E/F (for HW bisection)."""
    c = cfg
    K = kA + kB
    KH = c.C1 // 128                      # k-halves for layer-2 contraction
    H = c.HEADS
    nc = bacc.Bacc("TRN2", target_bir_lowering=False, debug=False,
                   num_devices=c.NCORES)

    # ---- IO ----
    x_own_d = nc.dram_tensor("x_own", [c.NBLK, 128, c.IN_C], BF, kind="ExternalInput").ap()
    w1_d = nc.dram_tensor("w1aug", [c.IN_C, c.C1 + 8], BF, kind="ExternalInput").ap()
    w2_d = nc.dram_tensor("w2aug", [128, KH, 72], BF, kind="ExternalInput").ap()
    idxA_d = nc.dram_tensor("idxA", [128, c.NBLK, kA * 8], I16, kind="ExternalInput").ap()
    idxB_d = nc.dram_tensor("idxB", [128, c.NBLK, kB * 8], I16, kind="ExternalInput").ap()
    dstrel_d = nc.dram_tensor("dstrel", [128, c.NBLK, K], BF, kind="ExternalInput").ap()
    iota_d = nc.dram_tensor("iota", [128, 128], BF, kind="ExternalInput").ap()
    eye_d = nc.dram_tensor("eye", [128, 128], BF, kind="ExternalInput").ap()
    ones1_d = nc.dram_tensor("ones1", [1, 128], BF, kind="ExternalInput").ap()
    ones_d = nc.dram_tensor("ones", [128, 1], F32, kind="ExternalInput").ap()
    e4_d = nc.dram_tensor("e4", [H, H * 128], BF, kind="ExternalInput").ap()
    pool_d = nc.dram_tensor("pool64", [c.OUT_C, 1], F32, kind="ExternalOutput").ap()
    if debug:
        h1dbg_d = nc.dram_tensor("h1dbg", [c.NDP, c.C1], F32, kind="ExternalOutput").ap()
        h2dbg_d = nc.dram_tensor("h2dbg", [c.NDP, 72], F32, kind="ExternalOutput").ap()

    # ---- internal DRAM ----
    t1own = nc.dram_tensor("t1own", [c.NDP, c.ROW1], BF, kind="Internal").ap()
    t1full = nc.dram_tensor("t1full", [c.RTOT, c.ROW1], BF, kind="Internal",
                            addr_space="Shared").ap()
    cc3in = nc.dram_tensor("cc3in", [c.NDP, c.ROW2], BF, kind="Internal").ap()
    cc3 = nc.dram_tensor("cc3", [c.RTOT, c.ROW2], BF, kind="Internal",
                         addr_space="Shared").ap()
    t3B = nc.dram_tensor("t3B", [c.RTOT - c.SPLIT, c.ROW2], BF, kind="Internal").ap()

    with tile.TileContext(nc) as tc:
        with tc.tile_pool(name="const", bufs=1) as cpool, \
             tc.tile_pool(name="pa", bufs=3) as pa, \
             tc.tile_pool(name="ppA", bufs=2, space="PSUM") as ppA, \
             tc.tile_pool(name="ppB", bufs=2, space="PSUM") as ppB, \
             tc.tile_pool(name="ppS", bufs=1, space="PSUM") as ppS, \
             tc.tile_pool(name="pgh", bufs=3) as pgh, \
             tc.tile_pool(name="pg", bufs=2) as pg, \
             tc.tile_pool(name="pt", bufs=1) as pt, \
             tc.tile_pool(name="pe2", bufs=2) as pe2, \
             tc.tile_pool(name="sm", bufs=3) as sm:

            # constants resident
            w1s = cpool.tile_from(w1_d)                     # [128, C1+8]
            w2s = cpool.tile_from(w2_d)                     # [128, KH, 72]
            iota_s = cpool.tile_from(iota_d)
            eye_s = cpool.tile_from(eye_d)
            ones1_s = cpool.tile_from(ones1_d)
            ones_s = cpool.tile_from(ones_d)
            e4_s = cpool.tile_from(e4_d)
            idxA_s = cpool.tile_from(idxA_d)
            idxB_s = cpool.tile_from(idxB_d)
            dstrel_s = cpool.tile_from(dstrel_d)

            hown_s = cpool.tile([128, c.NBLK, c.C1 + 8], BF)   # own h + alphas
            eeself_s = cpool.tile([128, c.NBLK, H], F32)
            c3own_s = cpool.tile([128, c.NBLK, 72], BF)        # own h2pre+alphas
            eeself2_s = cpool.tile([128, c.NBLK, 1], F32)
            pacc = cpool.tile([128, c.OUT_C], F32)
            nc.vector.memset(pacc[:], 0.0)

            # ========= phase A: own-shard h tiles, then AllGather table ======
            for j in range(c.NBLK):
                xo = pa.tile([128, c.IN_C], BF, tag="xo")
                nc.sync.dma_start(out=xo[:], in_=x_own_d[j, :, :])
                pso = ppA.tile([128, c.C1 + 8], F32, tag="A")
                nc.tensor.matmul(out=pso[:], lhsT=xo[:], rhs=w1s[:],
                                 start=True, stop=True)
                nc.scalar.activation(out=hown_s[:, j, :], in_=pso[:], func=AF.Copy)
                nc.sync.dma_start(
                    out=t1own[j * 128:(j + 1) * 128, 0:c.C1 + 8],
                    in_=hown_s[:, j, :])

            nc.gpsimd.collective_compute(
                kind="AllGather", op=AL.bypass,
                replica_groups=[list(range(c.NCORES))],
                ins=[t1own[:, :]], outs=[t1full[:, :]])

            # self-loop weights for all blocks: exp(lrelu(as_own + ad_own))
            zs = sm.tile([128, c.NBLK, H], F32, tag="zs")
            nc.vector.tensor_tensor(
                out=zs[:], in0=hown_s[:, :, c.C1:c.C1 + H],
                in1=hown_s[:, :, c.C1 + 4:c.C1 + 4 + H], op=AL.add)
            lrs = sm.tile([128, c.NBLK, H], F32, tag="lrs")
            nc.vector.scalar_tensor_tensor(
                out=lrs[:], in0=zs[:], scalar=c.NEG, in1=zs[:],
                op0=AL.mult, op1=AL.max)
            nc.scalar.activation(out=eeself_s[:], in_=lrs[:], func=AF.Exp)

            # ===== phase B+C interleaved: layer-1 aggregation + h2_pre =======
            if stage >= "B":
                for i in range(3):   # pre-zero rotating hc pad columns
                    hcz = pa.tile([128, c.ROW2], BF, tag="hc")
                    nc.vector.memset(hcz[:], 0.0)
            for b in (range(c.NBLK) if stage >= "B" else []):
                hg = pgh.tile([128, K, c.ROW1], BF, tag="hg")
                nc.gpsimd.dma_gather(
                    out_ap=hg[:, 0:kA, :], in_ap=t1full[0:c.SPLIT, :],
                    idxs_ap=idxA_s[:, b, :], num_idxs=kA * 128,
                    num_idxs_reg=kA * 128, elem_size=c.ROW1, single_packet=False)
                nc.gpsimd.dma_gather(
                    out_ap=hg[:, kA:K, :], in_ap=t1full[c.SPLIT:c.RTOT, :],
                    idxs_ap=idxB_s[:, b, :], num_idxs=kB * 128,
                    num_idxs_reg=kB * 128, elem_size=c.ROW1, single_packet=False)

                mask = pg.tile([128, K, 128], BF, tag="mask")
                nc.vector.tensor_tensor(
                    out=mask[:],
                    in0=iota_s[:, None, :].to_broadcast([128, K, 128]),
                    in1=dstrel_s[:, b, :, None].to_broadcast([128, K, 128]),
                    op=AL.is_equal)

                # dst-alpha broadcast: adT = own_ad^T, rank-1 replicate per head
                adTf = ppS.tile([128, 128], BF, tag="adT")
                adTp = adTf[0:H, :]
                nc.tensor.transpose(
                    out=adTp[:], in_=hown_s[:, b, c.C1 + 4:c.C1 + 4 + H],
                    identity=eye_s[:])
                adT = sm.tile([H, 128], BF, tag="adTs")
                nc.scalar.activation(out=adT[:], in_=adTp[:], func=AF.Copy)
                adrp = ppS.tile([128, H, 128], F32, tag="adrp")
                for h in range(H):
                    nc.tensor.matmul(out=adrp[:, h, :],
                                     lhsT=e4_s[:, h * 128:(h + 1) * 128],
                                     rhs=adT[:], start=True, stop=True)
                # head-pair-packed replicate: two bf16 ad values live in one
                # fp32 carrier; 1.0*x and 0+x are exact, so the one-hot
                # select preserves bits (carrier exponent byte is the bf16
                # exponent -> always a normal fp32).
                adrepP = sm.tile([128, 256], F32, tag="adrep")
                bfv = adrepP[:].bitcast(BF)
                nc.scalar.activation(
                    out=bfv.rearrange("p (c d w) -> p c w d", c=2, d=128, w=2),
                    in_=adrp[:].rearrange("p (c w) d -> p c w d", c=2, w=2),
                    func=AF.Copy)

                # per-edge dst alpha: zad[p,j,h] = sum_d mask[p,j,d]*adrep[p,h,d]
                tselP = pt.tile([128, K, 2, 128], F32, tag="tsel")
                nc.vector.tensor_tensor(
                    out=tselP[:],
                    in0=mask[:, :, None, :].to_broadcast([128, K, 2, 128]),
                    in1=adrepP[:].rearrange("p (c d) -> p c d", c=2)[
                        :, None, :, :].to_broadcast([128, K, 2, 128]),
                    op=AL.mult)
                zadP = sm.tile([128, K, 2], F32, tag="zad")
                nc.vector.tensor_reduce(out=zadP[:], in_=tselP[:], axis=AX.X,
                                        op=AL.add)

                z = sm.tile([128, K, H], F32, tag="z")
                nc.vector.tensor_tensor(
                    out=z[:], in0=hg[:, :, c.C1:c.C1 + H],
                    in1=zadP[:].bitcast(BF), op=AL.add)
                lr = sm.tile([128, K, H], F32, tag="lr")
                nc.vector.scalar_tensor_tensor(
                    out=lr[:], in0=z[:], scalar=c.NEG, in1=z[:],
                    op0=AL.mult, op1=AL.max)
                eeb = sm.tile([128, K, H], BF, tag="eeb")
                nc.scalar.activation(out=eeb[:], in_=lr[:], func=AF.Exp)

                v = pg.tile([128, K, c.C1 + 4], BF, tag="v")
                nc.vector.tensor_tensor(
                    out=v[:, :, 0:c.C1].rearrange("p k (h q) -> p k h q", h=H),
                    in0=hg[:, :, 0:c.C1].rearrange("p k (h q) -> p k h q", h=H),
                    in1=eeb[:, :, :, None].to_broadcast([128, K, H, c.HID]),
                    op=AL.mult)
                nc.scalar.activation(out=v[:, :, c.C1:c.C1 + 4], in_=eeb[:],
                                     func=AF.Copy)

                # self-loop row: vself[p,:] = eeself[p]*[h_own | 1] folded into
                # the aggregation as an extra identity-mask matmul
                vself = sm.tile([128, c.C1 + 4], BF, tag="vself")
                for h in range(H):
                    nc.scalar.activation(
                        out=vself[:, h * c.HID:(h + 1) * c.HID],
                        in_=hown_s[:, b, h * c.HID:(h + 1) * c.HID],
                        func=AF.Copy, scale=eeself_s[:, b, h:h + 1])
                nc.scalar.activation(out=vself[:, c.C1:c.C1 + 4],
                                     in_=eeself_s[:, b, :], func=AF.Copy)

                ps = ppB.tile([128, c.C1 + 4], F32, tag="B")
                for j in range(K):
                    nc.tensor.matmul(out=ps[:], lhsT=mask[:, j, :],
                                     rhs=v[:, j, :],
                                     start=(j == 0), stop=False)
                nc.tensor.matmul(out=ps[:], lhsT=eye_s[:], rhs=vself[:],
                                 start=False, stop=True)

                sbn = sm.tile([128, c.C1 + 4], F32, tag="sbn")
                nc.scalar.activation(out=sbn[:], in_=ps[:], func=AF.Copy)
                rec = sm.tile([128, H], F32, tag="rec")
                nc.vector.reciprocal(out=rec[:], in_=sbn[:, c.C1:c.C1 + 4])
                h1b = sm.tile([128, c.C1], BF, tag="h1b")
                nc.vector.scalar_tensor_tensor(
                    out=h1b[:].rearrange("p (h q) -> p h q", h=H),
                    in0=sbn[:, 0:c.C1].rearrange("p (h q) -> p h q", h=H),
                    scalar=0.0, op0=AL.max,
                    in1=rec[:, :, None].to_broadcast([128, H, c.HID]),
                    op1=AL.mult)
                if debug:
                    h1f = sm.tile([128, c.C1], F32, tag="h1f")
                    nc.vector.scalar_tensor_tensor(
                        out=h1f[:].rearrange("p (h q) -> p h q", h=H),
                        in0=sbn[:, 0:c.C1].rearrange("p (h q) -> p h q", h=H),
                        scalar=0.0, op0=AL.max,
                        in1=rec[:, :, None].to_broadcast([128, H, c.HID]),
                        op1=AL.mult)
                    nc.sync.dma_start(out=h1dbg_d[b * 128:(b + 1) * 128, :],
                                      in_=h1f[:])

                # ---- phase C for this block: h2_pre = h1 @ W2aug ----
                if stage >= "C":
                    pscf = ppA.tile([128, c.C1 + 8], F32, tag="A")
                    psc = pscf[:, 0:72]
                    for kh in range(KH):
                        htf = ppS.tile([128, 128], BF, tag="adT")
                        nc.tensor.transpose(
                            out=htf[:], in_=h1b[:, kh * 128:(kh + 1) * 128],
                            identity=eye_s[:])
                        ht = pa.tile([128, 128], BF, tag="ht")
                        nc.scalar.activation(out=ht[:], in_=htf[:], func=AF.Copy)
                        nc.tensor.matmul(out=psc[:], lhsT=ht[:], rhs=w2s[:, kh, :],
                                         start=(kh == 0), stop=(kh == KH - 1))
                    hc = pa.tile([128, c.ROW2], BF, tag="hc")
                    nc.scalar.activation(out=hc[:, 0:72], in_=psc[:], func=AF.Copy)
                    nc.scalar.activation(out=c3own_s[:, b, :], in_=hc[:, 0:72],
                                         func=AF.Copy)
                    nc.sync.dma_start(out=cc3in[b * 128:(b + 1) * 128, :], in_=hc[:])
                    if debug:
                        h2f = pa.tile([128, 72], F32, tag="h2f")
                        nc.vector.tensor_copy(out=h2f[:], in_=psc[:])
                        nc.sync.dma_start(out=h2dbg_d[b * 128:(b + 1) * 128, :],
                                          in_=h2f[:])

            if stage >= "C":
                zs2 = sm.tile([128, c.NBLK, 1], F32, tag="zs2")
                nc.vector.tensor_tensor(
                    out=zs2[:], in0=c3own_s[:, :, c.OUT_C:c.OUT_C + 1],
                    in1=c3own_s[:, :, c.OUT_C + 1:c.OUT_C + 2], op=AL.add)
                lrs2 = sm.tile([128, c.NBLK, 1], F32, tag="lrs2")
                nc.vector.scalar_tensor_tensor(
                    out=lrs2[:], in0=zs2[:], scalar=c.NEG, in1=zs2[:],
                    op0=AL.mult, op1=AL.max)
                nc.scalar.activation(out=eeself2_s[:], in_=lrs2[:], func=AF.Exp)

            # ================= phase D: allgather + repack ===================
            if stage >= "D":
                nc.gpsimd.collective_compute(
                    kind="AllGather", op=AL.bypass,
                    replica_groups=[list(range(c.NCORES))],
                    ins=[cc3in[:, :]], outs=[cc3[:, :]])
                nc.sync.dma_start(out=t3B[:, :], in_=cc3[c.SPLIT:c.RTOT, :])

            # ================= phase E: layer-2 edge aggregation =============
            for b in (range(c.NBLK) if stage >= "E" else []):
                hg2 = pe2.tile([128, K, c.ROW2], BF, tag="hg2")
                nc.gpsimd.dma_gather(
                    out_ap=hg2[:, 0:kA, :], in_ap=cc3[:, :],
                    idxs_ap=idxA_s[:, b, :], num_idxs=kA * 128,
                    num_idxs_reg=kA * 128, elem_size=c.ROW2, single_packet=False)
                nc.gpsimd.dma_gather(
                    out_ap=hg2[:, kA:K, :], in_ap=t3B[:, :],
                    idxs_ap=idxB_s[:, b, :], num_idxs=kB * 128,
                    num_idxs_reg=kB * 128, elem_size=c.ROW2, single_packet=False)

                mask2 = pe2.tile([128, K, 128], BF, tag="mask2")
                nc.vector.tensor_tensor(
                    out=mask2[:],
                    in0=iota_s[:, None, :].to_broadcast([128, K, 128]),
                    in1=dstrel_s[:, b, :, None].to_broadcast([128, K, 128]),
                    op=AL.is_equal)

                adTp2 = ppS.tile([1, 128], BF, tag="adT2")
                nc.tensor.transpose(
                    out=adTp2[:], in_=c3own_s[:, b, c.OUT_C + 1:c.OUT_C + 2],
                    identity=eye_s[:])
                adT2 = sm.tile([1, 128], BF, tag="adT2s")
                nc.scalar.activation(out=adT2[:], in_=adTp2[:], func=AF.Copy)
                adrp2 = ppS.tile([128, 128], F32, tag="adrp2")
                nc.tensor.matmul(out=adrp2[:], lhsT=ones1_s[:], rhs=adT2[:],
                                 start=True, stop=True)
                adrep2 = sm.tile([128, 128], BF, tag="adrep2")
                nc.scalar.activation(out=adrep2[:], in_=adrp2[:], func=AF.Copy)

                tsel2 = pt.tile([128, K, 128], BF, tag="tsel2")
                nc.vector.tensor_tensor(
                    out=tsel2[:], in0=mask2[:],
                    in1=adrep2[:, None, :].to_broadcast([128, K, 128]),
                    op=AL.mult)
                zad2 = sm.tile([128, K, 1], BF, tag="zad2")
                with nc.allow_low_precision("one-hot select, sum is exact"):
                    nc.vector.tensor_reduce(out=zad2[:], in_=tsel2[:], axis=AX.X,
                                            op=AL.add)

                z2 = sm.tile([128, K, 1], F32, tag="z2")
                nc.vector.tensor_tensor(
                    out=z2[:], in0=hg2[:, :, c.OUT_C:c.OUT_C + 1], in1=zad2[:],
                    op=AL.add)
                lr2 = sm.tile([128, K, 1], F32, tag="lr2")
                nc.vector.scalar_tensor_tensor(
                    out=lr2[:], in0=z2[:], scalar=c.NEG, in1=z2[:],
                    op0=AL.mult, op1=AL.max)
                ee2 = sm.tile([128, K, 1], BF, tag="ee2")
                nc.scalar.activation(out=ee2[:], in_=lr2[:], func=AF.Exp)

                v2 = pe2.tile([128, K, c.OUT_C + 1], BF, tag="v2")
                nc.vector.tensor_tensor(
                    out=v2[:, :, 0:c.OUT_C], in0=hg2[:, :, 0:c.OUT_C],
                    in1=ee2[:, :, 0:1].to_broadcast([128, K, c.OUT_C]),
                    op=AL.mult)
                nc.scalar.activation(out=v2[:, :, c.OUT_C:c.OUT_C + 1],
                                     in_=ee2[:], func=AF.Copy)

                vself2 = sm.tile([128, c.OUT_C + 1], BF, tag="vself2")
                nc.scalar.activation(
                    out=vself2[:, 0:c.OUT_C], in_=c3own_s[:, b, 0:c.OUT_C],
                    func=AF.Copy, scale=eeself2_s[:, b, 0:1])
                nc.scalar.activation(out=vself2[:, c.OUT_C:c.OUT_C + 1],
                                     in_=eeself2_s[:, b, :], func=AF.Copy)

                ps2f = ppB.tile([128, c.C1 + 4], F32, tag="B")
                ps2 = ps2f[:, 0:c.OUT_C + 1]
                for j in range(K):
                    nc.tensor.matmul(out=ps2[:], lhsT=mask2[:, j, :],
                                     rhs=v2[:, j, :],
                                     start=(j == 0), stop=False)
                nc.tensor.matmul(out=ps2[:], lhsT=eye_s[:], rhs=vself2[:],
                                 start=False, stop=True)

                sbn2 = sm.tile([128, c.OUT_C + 1], F32, tag="sbn2")
                nc.scalar.activation(out=sbn2[:], in_=ps2[:], func=AF.Copy)
                rec2 = sm.tile([128, 1], F32, tag="rec2")
                nc.vector.reciprocal(out=rec2[:], in_=sbn2[:, c.OUT_C:c.OUT_C + 1])
                o2 = sm.tile([128, c.OUT_C], F32, tag="o2")
                nc.vector.scalar_tensor_tensor(
                    out=o2[:], in0=sbn2[:, 0:c.OUT_C], scalar=0.0, op0=AL.max,
                    in1=rec2[:, 0:1].to_broadcast([128, c.OUT_C]), op1=AL.mult)
                nc.vector.tensor_tensor(out=pacc[:], in0=pacc[:], in1=o2[:],
                                        op=AL.add)

            # ================= phase F: pool partial =========================
            psff = ppB.tile([128, c.C1 + 4], F32, tag="B")
            psf = psff[0:c.OUT_C, 0:1]
            nc.tensor.matmul(out=psf[:], lhsT=pacc[:], rhs=ones_s[:],
                             start=True, stop=True)
            pf = sm.tile([c.OUT_C, 1], F32, tag="pf")
            nc.scalar.activation(out=pf[:], in_=psf[:], func=AF.Copy)
            nc.sync.dma_start(out=pool_d[:, :], in_=pf[:])

    nc.compile()
    legalize_waits(nc)
    return nc


def legalize_waits(nc):
    """Walrus encodes at most ONE sync wait per instruction on this toolchain.
    Hoist excess waits onto same-engine NoOps inserted before the instruction."""
    for fn in nc.m.functions:
        for bb in fn.blocks:
            insts = list(bb.instructions)
            out = []
            changed = False
            for inst in insts:
                si = inst.sync_info
                if si is not None and si.on_wait and len(si.on_wait) > 1:
                    waits = list(si.on_wait)
                    for w in waits[:-1]:
                        nop = mybir.InstNoOp(
                            name=nc.get_next_instruction_name(), ins=[], outs=[])
                        nop.engine = inst.engine
                        nop.sync_info = mybir.SyncInfo(on_wait=[w], on_update=[])
                        nc.register_instruction(nop)
                        out.append(nop)
                    inst.sync_info = mybir.SyncInfo(
                        on_wait=waits[-1:], on_update=list(si.on_update))
                    changed = True
                out.append(inst)
            if changed:
                bb.instructions.clear()
                bb.instructions.extend(out)


def host_finish(cfg, pools, fc_w, fc_b):
    c = cfg
    tot = np.zeros(c.OUT_C, np.float64)
    for p in pools:
        tot += p[:, 0].astype(np.float64)
    pooled = (tot / c.N).astype(np.float32)
    logits = pooled @ np.asarray(fc_w, np.float32) + np.asarray(fc_b, np.float32)
    m = logits.max()
    ls = logits - (m + np.log(np.exp(logits - m).sum()))
    return ls.reshape(1, c.NCLS).astype(np.float32)


_BUILD_CACHE = {}


def run(cfg, inputs, debug=False, trace=False, **run_kwargs):
    in_maps, meta = host_prep(
        cfg, inputs["x"], inputs["edge_index"], inputs["W1"], inputs["att_src1"],
        inputs["att_dst1"], inputs["b1"], inputs["W2"], inputs["att_src2"],
        inputs["att_dst2"], inputs["b2"])
    stage = os.environ.get("KSTAGE", "F")
    key = (cfg.N, cfg.E, meta["kA"], meta["kB"], debug, stage)
    if key not in _BUILD_CACHE:
        _BUILD_CACHE[key] = build(cfg, meta["kA"], meta["kB"], debug=debug,
                                  stage=stage)
    nc = _BUILD_CACHE[key]
    res = bass_utils.run_bass_kernel_spmd(
        nc, in_maps, core_ids=list(range(cfg.NCORES)), trace=trace, **run_kwargs)
    out = host_finish(cfg, [r["pool64"] for r in res.results],
                      inputs["fc_w"], inputs["fc_b"])
    return out, res


def kernel(**inputs):
    cfg = Cfg()
    out, _ = run(cfg, inputs)
    return out
